# revision 13
# baseline (speedup 1.0000x reference)
"""BasicAttn Trainium2 kernel.

Full inputs: Q, K, V [2, 8, 4096, 64] fp32 (+ unused attn_mask).
Outputs (matching the reference nn.Module): (context [2,8,4096,64],
attn [2,8,4096,4096]), both fp32.

Sharding: batch*heads (16 pairs) split across 8 NeuronCores, 2 bh-pairs
per core; attention is fully local per core.

Per (bh) the kernel runs two pipelines over the 4096x4096 score matrix:
  stage2 ([k,q] layout): scoresT tiles -> exp -> feeds context matmul
          (contraction over k needs k on partitions).
  stage1 ([q,k] layout): scores -> exp (with free per-partition rowsum
          via accum_out) -> normalize (DVE tensor_scalar 2x) -> DMA out.
Matmuls run in float32r (tf32-class, 1 cycle/row at N=512).
"""

import numpy as np
from contextlib import ExitStack

import bass_rust
import concourse.bass as bass
import concourse.tile as tile
from concourse import mybir
from concourse.bass_utils import run_bass_kernel_spmd
from concourse.masks import make_identity
from concourse.tile_rust import add_dep_helper

F32 = mybir.dt.float32
F32R = mybir.dt.float32r
MMDT = mybir.dt.float16
EXP = mybir.ActivationFunctionType.Exp

B, H, S, DK = 2, 8, 4096, 64
N_CORES = 8
BH_PER_CORE = (B * H) // N_CORES  # 2
SCALE = 1.0 / 8.0  # 1/sqrt(64)

NT = S // 128  # 32 k/q tiles of 128
NG = S // 512  # 8 groups of 512

_wf_counter = [0]


def _split_waits(nc, limit=1):
    """walrus here rejects >1 sync-wait command per instruction; hoist
    extras onto standalone EventSemaphore instructions just before."""
    for f in nc.m.functions:
        for bb in f.blocks:
            insts = list(bb.instructions)
            if not any(
                i.sync_info is not None and len(i.sync_info.on_wait or []) > limit
                for i in insts
            ):
                continue
            new = []
            for inst in insts:
                si = inst.sync_info
                if si is not None and si.on_wait and len(si.on_wait) > limit:
                    waits = list(si.on_wait)
                    for w in waits[:-limit]:
                        _wf_counter[0] += 1
                        ev = mybir.InstEventSemaphore(
                            name=f"I-waitsplit-{_wf_counter[0]}", ins=[], outs=[]
                        )
                        ev.engine = inst.engine
                        ev.sync_info = bass_rust.SyncInfo(on_wait=[w], on_update=[])
                        nc.register_instruction(ev, overwrite=True)
                        new.append(ev)
                    si.on_wait = waits[-limit:]
                new.append(inst)
            bb.instructions = new


def build_program():
    nc = bass.Bass(trn_type="TRN2")
    q_ext = nc.dram_tensor("q", [BH_PER_CORE, S, DK], F32, kind="ExternalInput")
    k_ext = nc.dram_tensor("k", [BH_PER_CORE, S, DK], F32, kind="ExternalInput")
    v_ext = nc.dram_tensor("v", [BH_PER_CORE, S, DK], F32, kind="ExternalInput")
    attn_ext = nc.dram_tensor("attn", [BH_PER_CORE, S, S], F32, kind="ExternalOutput")
    ctx_ext = nc.dram_tensor("ctx", [BH_PER_CORE, S, DK], F32, kind="ExternalOutput")

    with tile.TileContext(nc) as tc, ExitStack() as ctx:
        const_pool = ctx.enter_context(tc.tile_pool(name="const", bufs=1))
        stage_pool = ctx.enter_context(tc.tile_pool(name="stage", bufs=2))
        qt_pool = ctx.enter_context(tc.tile_pool(name="qt", bufs=2))
        kt_pool = ctx.enter_context(tc.tile_pool(name="kt", bufs=2))
        vr_pool = ctx.enter_context(tc.tile_pool(name="vr", bufs=2))
        expT_pool = ctx.enter_context(tc.tile_pool(name="expT", bufs=4))
        exp1_pool = ctx.enter_context(tc.tile_pool(name="exp1", bufs=2))
        attn_pool = ctx.enter_context(tc.tile_pool(name="attn", bufs=2))
        rs_pool = ctx.enter_context(tc.tile_pool(name="rs", bufs=2))
        rsum_pool = ctx.enter_context(tc.tile_pool(name="rsum", bufs=2))
        recip_pool = ctx.enter_context(tc.tile_pool(name="recip", bufs=8))
        ctxT_pool = ctx.enter_context(tc.tile_pool(name="ctxT", bufs=2))
        ctxf_pool = ctx.enter_context(tc.tile_pool(name="ctxf", bufs=1))

        ps2_pool = ctx.enter_context(tc.tile_pool(name="ps2", bufs=2, space="PSUM"))
        ps1_pool = ctx.enter_context(tc.tile_pool(name="ps1", bufs=1, space="PSUM"))
        psctx_pool = ctx.enter_context(tc.tile_pool(name="psc", bufs=1, space="PSUM"))
        psct2_pool = ctx.enter_context(tc.tile_pool(name="psc2", bufs=1, space="PSUM"))

        ident = const_pool.tile([128, 128], F32)
        make_identity(nc, ident[:])

        # HAM heater: fp32r matmuls don't register as PE activity, so the
        # clock gate re-throttles to 1.2 GHz.  Sprinkle tiny bf16 matmuls
        # through the schedule to keep the PE at 2.4 GHz.  Output goes to
        # unused partitions of the ctx-accumulator bank (values ignored;
        # start=False never clears other elements' has_written bits).
        heat_a = const_pool.tile([128, 16], mybir.dt.bfloat16)
        heat_b = const_pool.tile([128, 128], mybir.dt.bfloat16)
        nc.vector.memset(heat_a[:], 0.25)
        nc.vector.memset(heat_b[:], 0.25)
        heat_state = {"psc": None, "last": None, "pending": None}

        def mm(*args, **kw):
            """matmul wrapper that anchors heaters into the PE stream: the
            scheduler would otherwise clump the dependency-free heaters,
            leaving >3.4us HAM-idle windows."""
            inst = nc.tensor.matmul(*args, **kw)
            if heat_state["pending"] is not None:
                add_dep_helper(
                    inst.ins, heat_state["pending"].ins, sync=False,
                    reason="pe stream after heater",
                )
                heat_state["pending"] = None
            heat_state["last"] = inst
            return inst

        def heat():
            if True or heat_state["psc"] is None:
                return
            h = nc.tensor.matmul(
                heat_state["psc"][64:80, 0:128],
                heat_a[:],
                heat_b[:],
                start=False,
                stop=False,
                skip_group_check=True,
            )
            if heat_state["last"] is not None:
                add_dep_helper(
                    h.ins, heat_state["last"].ins, sync=False,
                    reason="heater after pe stream",
                )
            heat_state["pending"] = h

        for bh in range(BH_PER_CORE):
            # ---------- prep: load Q,K,V; build QT/KT [64, 4096] f32r ----------
            qn = stage_pool.tile([128, NT, DK], F32, tag="stage")
            nc.sync.dma_start(qn[:], q_ext[bh].rearrange("(t p) d -> p t d", p=128))
            kn = stage_pool.tile([128, NT, DK], F32, tag="stage")
            nc.sync.dma_start(kn[:], k_ext[bh].rearrange("(t p) d -> p t d", p=128))
            vn = stage_pool.tile([128, NT, DK], F32, tag="stage")
            nc.sync.dma_start(vn[:], v_ext[bh].rearrange("(t p) d -> p t d", p=128))

            qt = qt_pool.tile([64, S], MMDT)
            kt = kt_pool.tile([64, S], MMDT)
            vr = vr_pool.tile([128, NT, DK], MMDT)
            nc.vector.tensor_copy(vr[:], vn[:])

            for src, dst in ((qn, qt), (kn, kt)):
                for grp in range(NG // 2):
                    pt = ps2_pool.tile([128, 1024], F32, tag="ps2")
                    for j in range(8):
                        t = grp * 8 + j
                        nc.tensor.transpose(
                            pt[0:64, j * 128 : (j + 1) * 128],
                            src[:, t, :],
                            ident[:],
                        )
                    nc.vector.tensor_copy(
                        dst[:, grp * 1024 : (grp + 1) * 1024], pt[0:64, :]
                    )

            ctx_full = ctxf_pool.tile([128, NT * DK], F32)

            # ---------- main loop over q-groups of 512 ----------
            for g in range(NG):
                # stage2: scoresT [k,q] -> exp -> context accumulation
                psc = psctx_pool.tile([128, 512], F32)
                heat_state["psc"] = psc
                for tp in range(NT // 2):
                    p2 = ps2_pool.tile([128, 1024], F32, tag="ps2")
                    et = expT_pool.tile([128, 1024], MMDT)
                    for u in range(2):
                        t = tp * 2 + u
                        mm(
                            p2[:, u * 512 : (u + 1) * 512],
                            kt[:, t * 128 : (t + 1) * 128],
                            qt[:, g * 512 : (g + 1) * 512],
                            start=True,
                            stop=True,
                        )
                    nc.scalar.activation(et[:], p2[:], EXP, scale=SCALE)
                    for u in range(2):
                        t = tp * 2 + u
                        mm(
                            psc[0:64, :],
                            vr[:, t, :],
                            et[:, u * 512 : (u + 1) * 512],
                            start=(t == 0),
                            stop=(t == NT - 1),
                            skip_group_check=True,
                        )
                    heat()

                # stage1: scores [q,k] -> exp+rowsum -> normalize -> DMA
                recips = []
                for j in range(4):
                    qb = g * 4 + j
                    row0 = qb * 128
                    e1 = exp1_pool.tile([128, S], F32)
                    rs = rs_pool.tile([128, 4], F32)
                    for h in range(4):
                        p1 = ps1_pool.tile([128, 1024], F32, tag="ps1")
                        for kk in range(2):
                            mm(
                                p1[:, kk * 512 : (kk + 1) * 512],
                                qt[:, row0 : row0 + 128],
                                kt[:, h * 1024 + kk * 512 : h * 1024 + (kk + 1) * 512],
                                start=True,
                                stop=True,
                            )
                        nc.scalar.activation(
                            e1[:, h * 1024 : (h + 1) * 1024],
                            p1[:],
                            EXP,
                            scale=SCALE,
                            accum_out=rs[:, h : h + 1],
                        )
                        heat()
                    rsum = rsum_pool.tile([128, 1], F32)
                    nc.vector.tensor_reduce(
                        rsum[:], rs[:], mybir.AxisListType.X, mybir.AluOpType.add
                    )
                    rc = recip_pool.tile([128, 1], F32)
                    nc.vector.reciprocal(rc[:], rsum[:])
                    recips.append(rc)

                    at = attn_pool.tile([128, S], F32)
                    nc.vector.tensor_scalar_mul(at[:], e1[:], rc[:])
                    nc.sync.dma_start(attn_ext[bh, row0 : row0 + 128, :], at[:])

                # ctx finish for this q-group
                ct = ctxT_pool.tile([64, 512], F32)
                nc.vector.tensor_copy(ct[:], psc[0:64, :])
                pc2 = psct2_pool.tile([128, 4 * DK], F32)
                for j in range(4):
                    nc.tensor.transpose(
                        pc2[:, j * DK : (j + 1) * DK],
                        ct[:, j * 128 : (j + 1) * 128],
                        ident[0:64, 0:64],
                    )
                for j in range(4):
                    qb = g * 4 + j
                    nc.vector.tensor_scalar_mul(
                        ctx_full[:, qb * DK : (qb + 1) * DK],
                        pc2[:, j * DK : (j + 1) * DK],
                        recips[j][:],
                    )

            nc.sync.dma_start(
                ctx_ext[bh].rearrange("(t p) d -> p t d", p=128),
                ctx_full[:].rearrange("p (t d) -> p t d", d=DK),
            )

    _split_waits(nc)
    return nc


_program_cache = {}


def _get_program():
    if "nc" not in _program_cache:
        _program_cache["nc"] = build_program()
    return _program_cache["nc"]


def kernel(Q, K, V, attn_mask=None, trace=False, return_results=False):
    Q = np.ascontiguousarray(Q, dtype=np.float32)
    K = np.ascontiguousarray(K, dtype=np.float32)
    V = np.ascontiguousarray(V, dtype=np.float32)
    b, h, s, dk = Q.shape
    assert (b, h, s, dk) == (B, H, S, DK)

    qf = Q.reshape(B * H, S, DK)
    kf = K.reshape(B * H, S, DK)
    vf = V.reshape(B * H, S, DK)

    in_maps = []
    for c in range(N_CORES):
        sl = slice(c * BH_PER_CORE, (c + 1) * BH_PER_CORE)
        in_maps.append({"q": qf[sl], "k": kf[sl], "v": vf[sl]})

    nc = _get_program()
    res = run_bass_kernel_spmd(
        nc, in_maps, core_ids=list(range(N_CORES)), trace=trace
    )

    attn = np.empty((B * H, S, S), dtype=np.float32)
    ctxo = np.empty((B * H, S, DK), dtype=np.float32)
    for c in range(N_CORES):
        sl = slice(c * BH_PER_CORE, (c + 1) * BH_PER_CORE)
        attn[sl] = res.results[c]["attn"]
        ctxo[sl] = res.results[c]["ctx"]

    context = ctxo.reshape(B, H, S, DK)
    attn = attn.reshape(B, H, S, S)
    if return_results:
        return (context, attn), res
    return (context, attn)


# revision 18
# speedup vs baseline: 1.0884x; 1.0884x over previous
"""BasicAttn Trainium2 kernel.

Full inputs: Q, K, V [2, 8, 4096, 64] fp32 (+ unused attn_mask).
Outputs (matching the reference nn.Module): (context [2,8,4096,64],
attn [2,8,4096,4096]), both fp32.

Sharding: batch*heads (16 pairs) across 8 NeuronCores, 2 bh per core.

Per bh, per q-group of 512 columns:
  stage2: scoresT tiles [128k, 512q] via row-tiled fp16 matmul pairs
          (d_k=64 contraction -> two matmuls run concurrently in the
          top/bottom halves of the PE array), exp on ACT -> expT (fp16),
          then the context matmul (k contraction split into two 64-row
          halves, also concurrent) accumulating [65, 512] in PSUM; V
          carries a ones-column so row 64 accumulates the softmax
          denominator for free.
  recip:  reciprocal of the rowsum [1,512], PE-transposed to per-qblk
          [128,1] scalars.
  stage1: attn output tiles [128q, k]: k in [0,2048) computed directly
          (row-tiled matmul pairs + ACT exp + DVE tensor_scalar
          normalize at 2x); k in [2048,4096) produced by PE-transposing
          the (still resident) expT tiles and normalizing during the
          PSUM->SBUF copy.  This balances ACT (exp passes) against PE
          (transposes) - the machine throttles the PE to ~1.2 GHz under
          sustained load, so both engines are near their budget.
"""

import numpy as np
from contextlib import ExitStack

import bass_rust
import concourse.bass as bass
import concourse.tile as tile
from concourse import mybir
from concourse.bass_utils import run_bass_kernel_spmd
from concourse.masks import make_identity

F32 = mybir.dt.float32
F16 = mybir.dt.float16
EXP = mybir.ActivationFunctionType.Exp

B, H, S, DK = 2, 8, 4096, 64
N_CORES = 8
BH_PER_CORE = (B * H) // N_CORES  # 2
SCALE = 1.0 / 8.0  # 1/sqrt(64)

NT = S // 128  # 32 tiles of 128
NG = S // 512  # 8 q-groups

_wf_counter = [0]


def _split_waits(nc, limit=1):
    """walrus here rejects >1 sync-wait command per instruction; hoist
    extras onto standalone EventSemaphore instructions just before."""
    for f in nc.m.functions:
        for bb in f.blocks:
            insts = list(bb.instructions)
            if not any(
                i.sync_info is not None and len(i.sync_info.on_wait or []) > limit
                for i in insts
            ):
                continue
            new = []
            for inst in insts:
                si = inst.sync_info
                if si is not None and si.on_wait and len(si.on_wait) > limit:
                    waits = list(si.on_wait)
                    for w in waits[:-limit]:
                        _wf_counter[0] += 1
                        ev = mybir.InstEventSemaphore(
                            name=f"I-waitsplit-{_wf_counter[0]}", ins=[], outs=[]
                        )
                        ev.engine = inst.engine
                        ev.sync_info = bass_rust.SyncInfo(on_wait=[w], on_update=[])
                        nc.register_instruction(ev, overwrite=True)
                        new.append(ev)
                    si.on_wait = waits[-limit:]
                new.append(inst)
            bb.instructions = new


def build_program():
    nc = bass.Bass(trn_type="TRN2")
    q_ext = nc.dram_tensor("q", [BH_PER_CORE, S, DK], F32, kind="ExternalInput")
    k_ext = nc.dram_tensor("k", [BH_PER_CORE, S, DK], F32, kind="ExternalInput")
    v_ext = nc.dram_tensor("v", [BH_PER_CORE, S, DK], F32, kind="ExternalInput")
    attn_ext = nc.dram_tensor("attn", [BH_PER_CORE, S, S], F32, kind="ExternalOutput")
    rs_dram = nc.dram_tensor("rsscratch", [BH_PER_CORE * NG, 512], F32)
    ctx_ext = nc.dram_tensor("ctx", [BH_PER_CORE, S, DK], F32, kind="ExternalOutput")

    with tile.TileContext(nc) as tc, ExitStack() as ctx:
        const_pool = ctx.enter_context(tc.tile_pool(name="const", bufs=1))
        stage_pool = ctx.enter_context(tc.tile_pool(name="stage", bufs=2))
        qt_pool = ctx.enter_context(tc.tile_pool(name="qt", bufs=2))
        kt_pool = ctx.enter_context(tc.tile_pool(name="kt", bufs=2))
        vr_pool = ctx.enter_context(tc.tile_pool(name="vr", bufs=2))
        expT_pool = ctx.enter_context(tc.tile_pool(name="expT", bufs=14))
        exp1_pool = ctx.enter_context(tc.tile_pool(name="exp1", bufs=2))
        attn_pool = ctx.enter_context(tc.tile_pool(name="attn", bufs=3))
        rq_pool = ctx.enter_context(tc.tile_pool(name="rq", bufs=2))
        ctxT_pool = ctx.enter_context(tc.tile_pool(name="ctxT", bufs=2))
        ctxf_pool = ctx.enter_context(tc.tile_pool(name="ctxf", bufs=1))

        ps2_pool = ctx.enter_context(tc.tile_pool(name="ps2", bufs=3, space="PSUM"))
        psctx_pool = ctx.enter_context(tc.tile_pool(name="psc", bufs=1, space="PSUM"))

        ident = const_pool.tile([128, 128], F32)
        make_identity(nc, ident[:])
        identh = const_pool.tile([128, 128], F16)
        nc.vector.tensor_copy(identh[:], ident[:])

        for bh in range(BH_PER_CORE):
            # ---- prep: load Q,K,V; build qt/kt [128, S] f16 (dup halves) ----
            qn = stage_pool.tile([128, NT, DK], F32, tag="stage")
            nc.sync.dma_start(qn[:], q_ext[bh].rearrange("(t p) d -> p t d", p=128))
            kn = stage_pool.tile([128, NT, DK], F32, tag="stage")
            nc.sync.dma_start(kn[:], k_ext[bh].rearrange("(t p) d -> p t d", p=128))
            vn = stage_pool.tile([128, NT, DK], F32, tag="stage")
            nc.sync.dma_start(vn[:], v_ext[bh].rearrange("(t p) d -> p t d", p=128))

            qt = qt_pool.tile([128, S], F16)
            kt = kt_pool.tile([128, S], F16)
            vr = vr_pool.tile([128, NT, DK + 1], F16)
            nc.vector.tensor_copy(vr[:, :, 0:DK], vn[:])
            nc.vector.memset(vr[:, :, DK : DK + 1], 1.0)

            for src, dst in ((qn, qt), (kn, kt)):
                for grp in range(4):
                    pt = ps2_pool.tile([128, 1024], F32, tag="ps2")
                    for j in range(8):
                        t = grp * 8 + j
                        nc.tensor.transpose(
                            pt[0:64, j * 128 : (j + 1) * 128],
                            src[:, t, :],
                            ident[:],
                        )
                    sl = slice(grp * 1024, (grp + 1) * 1024)
                    nc.vector.tensor_copy(dst[0:64, sl], pt[0:64, :])
                    nc.scalar.copy(dst[64:128, sl], pt[0:64, :])

            ctx_full = ctxf_pool.tile([128, NT * DK], F32)

            # ---- main loop over q-groups of 512 ----
            for g in range(NG):
                g0 = g * 512

                # stage2: scoresT -> expT -> ctx accumulation (+rowsum row)
                psc = psctx_pool.tile([128, 1024], F32)
                nc.vector.memset(psc[0 : DK + 1, :], 0.0)
                ets = []
                for tp in range(NT // 2):
                    p2 = ps2_pool.tile([128, 1024], F32, tag="ps2")
                    et = expT_pool.tile([128, 1024], F16)
                    ets.append(et)
                    for u in range(2):
                        t = tp * 2 + u
                        half = slice(u * 64, (u + 1) * 64)
                        nc.tensor.matmul(
                            p2[:, u * 512 : (u + 1) * 512],
                            kt[half, t * 128 : (t + 1) * 128],
                            qt[half, g0 : g0 + 512],
                            start=True,
                            stop=True,
                            tile_position=(u * 64, 0),
                        )
                    nc.scalar.activation(et[:], p2[:], EXP, scale=SCALE)
                    for u in range(2):
                        t = tp * 2 + u
                        for v in range(2):
                            nc.tensor.matmul(
                                psc[0 : DK + 1, v * 512 : (v + 1) * 512],
                                vr[v * 64 : (v + 1) * 64, t, :],
                                et[v * 64 : (v + 1) * 64, u * 512 : (u + 1) * 512],
                                start=False,
                                stop=(tp == NT // 2 - 1 and u == 1 and v == 1),
                                skip_group_check=True,
                                tile_position=(v * 64, 0),
                            )

                # combine the two contraction halves; rowsum is row 64
                cta = ctxT_pool.tile([DK + 1, 512], F32, tag="cta")
                nc.vector.tensor_copy(cta[:], psc[0 : DK + 1, 0:512])
                ct = ctxT_pool.tile([DK + 1, 512], F32, tag="ct")
                nc.vector.tensor_add(ct[:], cta[:], psc[0 : DK + 1, 512:1024])
                # transpose rowsum [1,512] -> [128,4] via tiny DMA, then recip
                rqs = rq_pool.tile([128, 4], F32, tag="rqs")
                nc.sync.dma_start(rs_dram[bh * NG + g], ct[DK : DK + 1, :])
                nc.sync.dma_start(
                    rqs[:], rs_dram[bh * NG + g].rearrange("(j p) -> p j", p=128)
                )
                rq = rq_pool.tile([128, 4], F32, tag="rq")
                nc.vector.reciprocal(rq[:], rqs[:])

                # ctx finish: transpose to [q, d], scale
                pc2 = ps2_pool.tile([128, 4 * DK], F32, tag="ps2")
                for j in range(4):
                    nc.tensor.transpose(
                        pc2[:, j * DK : (j + 1) * DK],
                        ct[0:DK, j * 128 : (j + 1) * 128],
                        ident[0:64, 0:64],
                    )
                for j in range(4):
                    qb = g * 4 + j
                    nc.vector.tensor_scalar_mul(
                        ctx_full[:, qb * DK : (qb + 1) * DK],
                        pc2[:, j * DK : (j + 1) * DK],
                        rq[:, j : j + 1],
                    )

                # stage1: attn tiles for the 4 qblks of this q-group
                for j in range(4):
                    qb = g * 4 + j
                    row0 = qb * 128
                    at = attn_pool.tile([128, S], F32)

                    # k in [0, 2048): direct matmul + exp + normalize
                    e1 = exp1_pool.tile([128, 2048], F32)
                    for h in range(2):
                        p1 = ps2_pool.tile([128, 1024], F32, tag="ps2")
                        for u in range(2):
                            ks = h * 1024 + u * 512
                            half = slice(u * 64, (u + 1) * 64)
                            nc.tensor.matmul(
                                p1[:, u * 512 : (u + 1) * 512],
                                qt[half, row0 : row0 + 128],
                                kt[half, ks : ks + 512],
                                start=True,
                                stop=True,
                                tile_position=(u * 64, 0),
                            )
                        nc.scalar.activation(
                            e1[:, h * 1024 : (h + 1) * 1024], p1[:], EXP, scale=SCALE
                        )
                    nc.vector.tensor_scalar_mul(
                        at[:, 0:2048], e1[:], rq[:, j : j + 1]
                    )

                    # k in [2048, 4096): transpose resident expT tiles
                    for h in range(2, 4):
                        pst = ps2_pool.tile([128, 1024], F16, tag="ps2")
                        for w in range(8):
                            t = h * 8 + w  # k-tile index 16..31
                            et = ets[t // 2]
                            c0 = (t % 2) * 512 + j * 128
                            nc.tensor.transpose(
                                pst[:, w * 128 : (w + 1) * 128],
                                et[:, c0 : c0 + 128],
                                identh[:],
                            )
                        nc.vector.tensor_scalar_mul(
                            at[:, h * 1024 : (h + 1) * 1024],
                            pst[:],
                            rq[:, j : j + 1],
                        )

                    nc.sync.dma_start(attn_ext[bh, row0 : row0 + 128, :], at[:])

            nc.sync.dma_start(
                ctx_ext[bh].rearrange("(t p) d -> p t d", p=128),
                ctx_full[:].rearrange("p (t d) -> p t d", d=DK),
            )

    _split_waits(nc)
    return nc


_program_cache = {}


def _get_program():
    if "nc" not in _program_cache:
        _program_cache["nc"] = build_program()
    return _program_cache["nc"]


def kernel(Q, K, V, attn_mask=None, trace=False, return_results=False):
    Q = np.ascontiguousarray(Q, dtype=np.float32)
    K = np.ascontiguousarray(K, dtype=np.float32)
    V = np.ascontiguousarray(V, dtype=np.float32)
    b, h, s, dk = Q.shape
    assert (b, h, s, dk) == (B, H, S, DK)

    qf = Q.reshape(B * H, S, DK)
    kf = K.reshape(B * H, S, DK)
    vf = V.reshape(B * H, S, DK)

    in_maps = []
    for c in range(N_CORES):
        sl = slice(c * BH_PER_CORE, (c + 1) * BH_PER_CORE)
        in_maps.append({"q": qf[sl], "k": kf[sl], "v": vf[sl]})

    nc = _get_program()
    res = run_bass_kernel_spmd(
        nc, in_maps, core_ids=list(range(N_CORES)), trace=trace
    )

    attn = np.empty((B * H, S, S), dtype=np.float32)
    ctxo = np.empty((B * H, S, DK), dtype=np.float32)
    for c in range(N_CORES):
        sl = slice(c * BH_PER_CORE, (c + 1) * BH_PER_CORE)
        attn[sl] = res.results[c]["attn"]
        ctxo[sl] = res.results[c]["ctx"]

    context = ctxo.reshape(B, H, S, DK)
    attn = attn.reshape(B, H, S, S)
    if return_results:
        return (context, attn), res
    return (context, attn)
